# revision 33
# baseline (speedup 1.0000x reference)
"""Stereo cost-volume + softmax disparity regression + bilinear upsample.

Full inputs:  feat_l, feat_r [16, 4, 128, 240] f32, img_h=1024, img_w=1920.
Full output:  [16, 1, 1024, 1920] f32.

Sharding: pure data parallel, 2 samples per core across 8 cores.

Per-core layout: SBUF partitions p = ch*32 + (y % 32); free dim packs
(y_block, x) = 4*240 = 960 columns per sample; the two samples are
processed as a software pipeline (sample 1's cost volume overlaps sample
0's upsample) so DVE-heavy phase 1 and copy-heavy phase 2 share the span.

Per sample:
  1. cost volume: fp16 DVE subtract against a left-zero-padded feat_r
     (two pad copies so even/odd disparity shifts stay 4B-aligned for the
     DVE 2x mode), abs in place (even d: DVE bitwise-and sign clear at 4x;
     odd d: ACT Abs), channel-sum via PE selector matmuls (col-tiled,
     4 disparities per pass) into 1-bank PSUM chunks.
  2. softmax regression: ACT exp(8-cost) -> fp16, PE matmul with
     [ones ; 8*d] weights accumulating (s, t) in PSUM f32 over all 24
     disparities, DVE reciprocal + multiply -> pred fp16.
  3. upsample (align_corners bilinear = two dense fp16 matmuls): DMA xbar
     transpose pred -> predT, M1: tmp[y, X] = predT.T @ WxT (X padded to
     2048 so all PSUM chunks are 512-col bank-aligned), M2: out[Y, X] =
     WyT_chunk.T @ tmp, PSUM -> SBUF copies alternating DVE/ACT, fp16 DMA
     to HBM, host casts to f32.

All matmul PSUM outputs are <= 512 f32 columns and bank-aligned (a 480-col
chunk at offset 480 would straddle the 2 KiB bank boundary and silently
corrupt). PSUM budget: cost/tmp shared 1-bank slots (2 banks) + s/t
accumulators (4) + out chunks (2) = 8 banks, so both pipeline stages hold
their PSUM concurrently.
"""

import sys

sys.path.insert(0, "/opt/trn_rl_repo")

import numpy as np

import concourse.bacc as bacc
import concourse.tile as tile
import concourse.mybir as mybir
from concourse.bass_utils import run_bass_kernel_spmd

# ---------------------------------------------------------------- constants
B, C, H0, W0 = 16, 4, 128, 240
D = 24             # disparities
NCORES = 8
SPC = B // NCORES  # samples per core = 2
HI, WI = 1024, 1920
WP = WI            # X chunked as 512,512,512,384 (bank-aligned starts)
XCH = [(0, 512), (512, 512), (1024, 512), (1536, 384)]
YB = H0 // 32      # 4 y-blocks
G = SPC * YB       # 8 feat groups (sample-major)
FREE = G * W0      # 1920
PAD = 28           # left-pad columns in padded feat_r groups (>= D+2, even)
GW = W0 + 2 * PAD  # padded group width (even)
EXP_BIAS = 8.0

FP16 = mybir.dt.float16
F32 = mybir.dt.float32
U16 = mybir.dt.uint16

_TRACE = [False]


# ------------------------------------------------------------- host weights
def _host_consts():
    # selector for channel sum: sel[ch*32+y32, m] = (y32 == m)
    sel = np.zeros((128, 32), np.float16)
    for ch in range(C):
        sel[ch * 32 : (ch + 1) * 32, :] = np.eye(32, dtype=np.float16)

    # s/t weights per disparity group g: stw[dj*32+y32, m]
    #   m in [0,32): s-selector (ones);  m in [32,64): t = 8*d selector
    stw = np.zeros((128, 6 * 64), np.float16)
    for g in range(6):
        for dj in range(4):
            d = 4 * g + dj
            blk = stw[dj * 32 : (dj + 1) * 32, g * 64 : (g + 1) * 64]
            blk[:, 0:32] = np.eye(32, dtype=np.float16)
            blk[:, 32:64] = np.eye(32, dtype=np.float16) * np.float16(8.0 * d)

    # x-interp weights wxT[x, X], f32 linspace to match jnp rounding
    xs = np.linspace(0.0, W0 - 1.0, WI, dtype=np.float32)
    x0 = np.floor(xs).astype(np.int64)
    x1 = np.minimum(x0 + 1, W0 - 1)
    wx = (xs - x0).astype(np.float32)
    wxT_full = np.zeros((W0, WI), np.float32)
    wxT_full[x0, np.arange(WI)] += 1.0 - wx
    wxT_full[x1, np.arange(WI)] += wx
    # two overlapping 128-row x-chunks (DMA transpose needs 128-col blocks);
    # the 16 overlap rows (x 112:128) are zeroed in chunk B
    wxT = np.zeros((256, WI), np.float32)
    wxT[0:128] = wxT_full[0:128]
    wxT[144:256] = wxT_full[128:240]

    # y-interp weights wyT[y, Y]
    ys = np.linspace(0.0, H0 - 1.0, HI, dtype=np.float32)
    y0 = np.floor(ys).astype(np.int64)
    y1 = np.minimum(y0 + 1, H0 - 1)
    wy = (ys - y0).astype(np.float32)
    wyT = np.zeros((H0, HI), np.float32)
    wyT[y0, np.arange(HI)] += 1.0 - wy
    wyT[y1, np.arange(HI)] += wy

    ident = np.eye(128, dtype=np.float16)
    return {
        "sel": sel,
        "stw": stw,
        "wxT": wxT.astype(np.float16),
        "wyT": wyT.astype(np.float16),
        "ident": ident,
    }


def _pack_feat(f):
    """[SPC, C, H0, W0] -> [128, FREE] with p=(ch,y32), free=(s,yb,x)."""
    a = f.reshape(SPC, C, YB, 32, W0)
    a = np.ascontiguousarray(a.transpose(1, 3, 0, 2, 4))  # ch,y32,s,yb,x
    return a.reshape(128, FREE)


# ------------------------------------------------------------- build kernel
def _build():
    nc = bacc.Bacc("TRN2", target_bir_lowering=False, debug=False,
                   num_devices=NCORES)
    lf = nc.dram_tensor("lf", [128, FREE], FP16, kind="ExternalInput").ap()
    rf = nc.dram_tensor("rf", [128, FREE], FP16, kind="ExternalInput").ap()
    sel_d = nc.dram_tensor("sel", [128, 32], FP16, kind="ExternalInput").ap()
    stw_d = nc.dram_tensor("stw", [128, 384], FP16, kind="ExternalInput").ap()
    wxT_d = nc.dram_tensor("wxT", [256, WI], FP16, kind="ExternalInput").ap()
    wyT_d = nc.dram_tensor("wyT", [H0, HI], FP16, kind="ExternalInput").ap()
    idn_d = nc.dram_tensor("ident", [128, 128], FP16,
                           kind="ExternalInput").ap()
    out = nc.dram_tensor("out", [SPC, HI, WI], FP16,
                         kind="ExternalOutput").ap()

    AF = mybir.ActivationFunctionType
    OP = mybir.AluOpType

    with tile.TileContext(nc) as tc:
        with (
            tc.tile_pool(name="consts", bufs=1) as consts,
            tc.tile_pool(name="feat", bufs=1) as feat,
            tc.tile_pool(name="diff", bufs=8) as diffp,
            tc.tile_pool(name="ep", bufs=6) as ep,
            tc.tile_pool(name="predp", bufs=1) as predp,
            tc.tile_pool(name="upsb", bufs=1) as upsb,
            tc.tile_pool(name="outsb", bufs=6) as outsb,
            # PSUM budget (8 banks): ps1 (cost/tmp share 1-bank slots) = 2,
            # st (2 samples x [64,1024] f32) = 4, out chunks = 2
            tc.tile_pool(name="ps1", bufs=2, space="PSUM") as ps1,
            tc.tile_pool(name="outps", bufs=2, space="PSUM") as outps,
        ):
            from contextlib import ExitStack
            st_stack = ExitStack()
            stps = st_stack.enter_context(
                tc.tile_pool(name="stps", bufs=1, space="PSUM"))
            # ---- features first (phase-1 critical path), parallel queues
            L = feat.tile([128, FREE], FP16)
            nc.sync.dma_start(out=L, in_=lf)
            L3 = L.rearrange("p (g w) -> p g w", w=W0)
            # padded feat_r, two copies for even/odd shift alignment
            rf3 = rf.rearrange("p (g w) -> p g w", w=W0)
            R = []  # R[par][h] -> [128, YB, GW] view
            for par in range(2):
                Rh = []
                for h2 in range(SPC):
                    Rt = feat.tile([128, YB * GW], FP16,
                                   tag=f"rpad{par}{h2}",
                                   name=f"rpad{par}{h2}")
                    nc.gpsimd.memset(Rt, 0.0)
                    Rv = Rt.rearrange("p (g w) -> p g w", w=GW)
                    dma_eng = nc.scalar if par == 0 else nc.gpsimd
                    dma_eng.dma_start(
                        out=Rv[:, :, PAD + par : PAD + par + W0],
                        in_=rf3[:, YB * h2 : YB * h2 + YB, :],
                    )
                    Rh.append(Rv)
                R.append(Rh)

            # ---- constants (needed a bit later) on the scalar queue
            sel = consts.tile([128, 32], FP16)
            nc.gpsimd.dma_start(out=sel, in_=sel_d)
            stw = consts.tile([128, 384], FP16)
            nc.gpsimd.dma_start(out=stw, in_=stw_d)
            wxT = [consts.tile([128, WI], FP16, name=f"wxT{i}", tag=f"wxT{i}")
                   for i in range(2)]
            nc.gpsimd.dma_start(out=wxT[0], in_=wxT_d[0:128, :])
            nc.gpsimd.dma_start(out=wxT[1], in_=wxT_d[128:256, :])
            wyT = consts.tile([128, HI], FP16)
            nc.gpsimd.dma_start(out=wyT, in_=wyT_d)
            idn = consts.tile([128, 128], FP16)
            nc.gpsimd.dma_start(out=idn, in_=idn_d)
            bias8 = consts.tile([128, 1], F32)
            nc.vector.memset(bias8, EXP_BIAS)

            st = [stps.tile([64, 1024], F32, name=f"st{h}", tag=f"st{h}")
                  for h in range(SPC)]

            copy_tick = [0]

            def psum_copy(dst, src, dve_mod=2):
                if copy_tick[0] % dve_mod == 0:
                    nc.vector.tensor_copy(out=dst, in_=src)
                else:
                    nc.scalar.copy(out=dst, in_=src)
                copy_tick[0] += 1

            # ============ software pipeline over the two samples =========
            # Engine instruction streams are in-order, so sample 0's
            # upsample work is interleaved with sample 1's cost volume at
            # emission time to avoid head-of-line blocking.
            pred = [None, None]

            def emit_ph1_group(h, g):
                gs = slice(YB * h, YB * h + YB)
                absd = []
                for dj in range(4):
                    d = 4 * g + dj
                    par = d % 2
                    off = PAD + par - d
                    diff = diffp.tile([128, YB, W0], FP16, name="diff",
                                      tag="diff")
                    nc.vector.tensor_tensor(
                        out=diff, in0=L3[:, gs, :],
                        in1=R[par][h][:, :, off : off + W0],
                        op=OP.subtract,
                    )
                    # sample 0 runs alone (ACT idle): split abs by parity;
                    # sample 1 overlaps sample 0's copies: abs all on DVE
                    if h == 0 and d % 2 == 1:
                        nc.scalar.activation(out=diff, in_=diff, func=AF.Abs)
                    else:
                        di = diff.bitcast(U16)
                        nc.vector.tensor_scalar(
                            out=di, in0=di, scalar1=0x7FFF, scalar2=None,
                            op0=OP.bitwise_and,
                        )
                    absd.append(diff.rearrange("p g w -> p (g w)"))
                e = ep.tile([128, 1024], FP16, name="e", tag="e")
                for nch in range(2):
                    cost = ps1.tile([128, 512], F32, name="cost", tag="ps1")
                    for dj in range(4):
                        nc.tensor.matmul(
                            cost[dj * 32 : dj * 32 + 32, 0:480],
                            lhsT=sel,
                            rhs=absd[dj][:, nch * 480 : nch * 480 + 480],
                            start=True, stop=True,
                            tile_position=(0, dj * 32),
                        )
                    nc.scalar.activation(
                        out=e[:, nch * 512 : nch * 512 + 480],
                        in_=cost[:, 0:480], func=AF.Exp,
                        bias=bias8, scale=-1.0)
                for nch in range(2):
                    nc.tensor.matmul(
                        st[h][0:64, nch * 512 : nch * 512 + 480],
                        lhsT=stw[:, g * 64 : g * 64 + 64],
                        rhs=e[:, nch * 512 : nch * 512 + 480],
                        start=(g == 0), stop=(g == 5),
                        tile_position=(0, 0),
                        skip_group_check=True,
                    )

            def emit_pred(h):
                rs = predp.tile([32, 1024], F32, name=f"rs{h}", tag=f"rs{h}")
                pr = predp.tile([32, 1024], FP16, name=f"pred{h}",
                                tag=f"pred{h}")
                for nch in range(2):
                    sl = slice(nch * 512, nch * 512 + 480)
                    nc.vector.reciprocal(out=rs[:, sl], in_=st[h][0:32, sl])
                    nc.vector.tensor_tensor(out=pr[:, sl],
                                            in0=st[h][32:64, sl],
                                            in1=rs[:, sl], op=OP.mult)
                pred[h] = pr

            def emit_ph2_head(h, dve_mod, pool=None):
                """transposes + M1 -> tmp_sb for sample h"""
                pr = pred[h]
                predT = []
                for xh in range(2):
                    pt_ps = (pool() if pool else
                             ps1.tile([128, 512], F32, name="pt_ps",
                                      tag="ps1")).bitcast(FP16)
                    for yb in range(YB):
                        pcol = (yb // 2) * 512 + (yb % 2) * W0
                        nc.tensor.transpose(
                            pt_ps[0:128, yb * 32 : yb * 32 + 32],
                            pr[0:32, pcol + xh * 112 :
                               pcol + xh * 112 + 128],
                            idn[0:32, 0:32],
                        )
                    pt = upsb.tile([128, 128], FP16, tag=f"predT{h}{xh}",
                                   name=f"predT{h}{xh}")
                    nc.scalar.copy(out=pt, in_=pt_ps[0:128, 0:128])
                    predT.append(pt)
                tmp_sb = upsb.tile([128, WP], FP16, tag=f"tmp{h}",
                                   name=f"tmp{h}")
                for c0, nw in XCH:
                    t_ps = (pool() if pool else
                            ps1.tile([128, 512], F32, name="t_ps",
                                     tag="ps1"))
                    for xh in range(2):
                        nc.tensor.matmul(
                            t_ps[:, 0:nw], lhsT=predT[xh],
                            rhs=wxT[xh][:, c0 : c0 + nw],
                            start=(xh == 0), stop=(xh == 1),
                        )
                    psum_copy(tmp_sb[:, c0 : c0 + nw], t_ps[:, 0:nw],
                              dve_mod)
                return tmp_sb

            dma_tick = [0]

            def emit_ph2_yc(h, tmp_sb, yc, dve_mod, pool=None):
                ob = outsb.tile([128, WP], FP16, name="ob", tag="ob")
                for c0, nw in XCH:
                    o_ps = (pool() if pool else
                            outps.tile([128, 512], F32, name="o_ps",
                                       tag="o_ps"))
                    nc.tensor.matmul(
                        o_ps[:, 0:nw],
                        lhsT=wyT[:, yc * 128 : yc * 128 + 128],
                        rhs=tmp_sb[:, c0 : c0 + nw],
                        start=True, stop=True,
                    )
                    psum_copy(ob[:, c0 : c0 + nw], o_ps[:, 0:nw], dve_mod)
                eng = nc.sync if dma_tick[0] % 2 == 0 else nc.gpsimd
                dma_tick[0] += 1
                eng.dma_start(
                    out=out[h, yc * 128 : yc * 128 + 128, :],
                    in_=ob)

            # sample 0 cost volume + regression
            for g in range(6):
                emit_ph1_group(0, g)
            emit_pred(0)
            # interleave: sample 1 phase 1 with sample 0 upsample.
            # s1's first groups go first so DVE has runway while s0's
            # transposes/M1 chain resolves; head copies stay off DVE.
            emit_ph1_group(1, 0)
            tmp0 = emit_ph2_head(0, dve_mod=10**9)
            s0_yc = 0
            for g in range(1, 6):
                emit_ph1_group(1, g)
                if s0_yc < 5:
                    emit_ph2_yc(0, tmp0, s0_yc, dve_mod=5)
                    s0_yc += 1
            emit_pred(1)
            # finish sample 0 on the existing pools (no WAR on st banks)
            while s0_yc < 8:
                emit_ph2_yc(0, tmp0, s0_yc, dve_mod=2)
                s0_yc += 1
            tmp1 = emit_ph2_head(1, dve_mod=2)
            st_stack.close()  # free the 4 s/t banks for the tail
            with tc.tile_pool(name="pstail", bufs=4, space="PSUM") as pstail:
                tailps = [pstail]

                def tail_tile():
                    return tailps[0].tile([128, 512], F32, name="tl",
                                          tag="tl")

                for yc in range(8):
                    emit_ph2_yc(1, tmp1, yc, dve_mod=2, pool=tail_tile)
    nc.compile()
    return nc


_NC_CACHE = [None]


def kernel(feat_l, feat_r, img_h, img_w):
    feat_l = np.asarray(feat_l, dtype=np.float32)
    feat_r = np.asarray(feat_r, dtype=np.float32)
    assert int(img_h) == HI and int(img_w) == WI
    assert feat_l.shape == (B, C, H0, W0)

    if _NC_CACHE[0] is None:
        _NC_CACHE[0] = _build()
    nc = _NC_CACHE[0]

    consts = _host_consts()
    in_maps = []
    for c in range(NCORES):
        fl = _pack_feat(feat_l[SPC * c : SPC * c + SPC].astype(np.float16))
        fr = _pack_feat(feat_r[SPC * c : SPC * c + SPC].astype(np.float16))
        in_maps.append({"lf": fl, "rf": fr, **consts})

    res = run_bass_kernel_spmd(nc, in_maps, core_ids=list(range(NCORES)),
                               trace=_TRACE[0])
    outs = [res.results[i]["out"].astype(np.float32) for i in range(NCORES)]
    full = np.concatenate(outs, axis=0).reshape(B, 1, HI, WI)
    kernel._last_exec_ns = res.exec_time_ns
    return full
